# revision 40
# baseline (speedup 1.0000x reference)
"""Causal single-head attention on 8 Trainium2 NeuronCores.

Problem: x[8, 2048, 1024] -> out[8, 2048, 64]
  q/k/v = x @ W{q,k,v} + b{q,k,v};  out = softmax(causal(q k^T / 8)) v

Sharding: data-parallel over batch; core b computes batch element b.

Per-core design (T=2048, D=1024, H=64), all matmul operands bf16 with
fp32 PSUM accumulation:
  - host packs ONE bf16 blob [128, 18080] = wqk | wv | ident | vtail | x^T
    plus a [128, 2] f32 bias blob; 8 merged DMA issues (bias, wqk, x
    quarter 0 in two halves, wv+ident+vtail, x quarters 1-3) so the
    first QKV matmul starts as early as possible.
  - QKV per 512-col t-group: lhsT = wqk chunk [128d, 128] -> qT/kT
    [64, T]; V is COL-TILED: two M=64 matmuls at tile_position (0,0)
    and (0,64) compute v^T for the two 256-col halves of a quarter
    concurrently on disjoint PE column groups.
  - v^T tiles PE-transposed in bf16 (cheap) to natural v [128t, 64h]
    + ones/zeros columns -> v_sb [128, 16, 66].
  - attention in jt-PAIR rounds per i-quarter (512 wide): S^T for two
    j-chunks -> one 2-bank PSUM tile [128, 1024]; ONE exp ACTIVATE per
    round (scale=1/8 fused) -> P bf16; causal diagonal blocks masked by
    affine_select; PV accumulates out^T[66, 512] per quarter, whose
    row 64 is the softmax denominator (ones column of v_sb).
  - drain per quarter: out^T cast to bf16, PE-transposed (bf16) to
    natural [128, 4, 66] PSUM, reciprocal of row 64, per-t-tile scale,
    DMA out. Drains and later-quarter QKV work run as fillers inside
    earlier rounds to keep the PE busy during exp waits.
"""

import os
from contextlib import ExitStack

import ml_dtypes
import numpy as np

import concourse.bacc as bacc
import concourse.mybir as mybir
import concourse.tile as tile
from concourse.bass_utils import run_bass_kernel_spmd

F32 = mybir.dt.float32
BF16 = mybir.dt.bfloat16
AF = mybir.ActivationFunctionType
ALU = mybir.AluOpType

T = 2048
D = 1024
H = 64
NB = 8
DC = D // 128       # 8 contraction chunks
NJT = T // 128      # 16 j-chunks (also 16 t-tiles)
QW = 512            # i-quarter width
NQ = T // QW        # 4 quarters
SCALE = 1.0 / 8.0   # 1/sqrt(H)

# blob column offsets (bf16 elements per partition)
OFF_WQK = 0            # [128, 8, 128] -> 1024
OFF_WV = 1024          # [128, 8, 64]  -> 512
OFF_ID = 1536          # [128, 128]    -> 128
OFF_VT = 1664          # [128, 16, 2]  -> 32
OFF_X = 1696           # [128, 8, 2048] -> 16384
BLOB_W = OFF_X + DC * T

_CACHE: dict = {}


def _build():
    nc = bacc.Bacc("TRN2", target_bir_lowering=False, debug=False,
                   num_devices=NB)
    wx = nc.dram_tensor("wx", [128, BLOB_W], BF16, kind="ExternalInput")
    bias2 = nc.dram_tensor("bias2", [128, 2], F32, kind="ExternalInput")
    out = nc.dram_tensor("out", [T, H], F32, kind="ExternalOutput")

    with ExitStack() as ctx:
        tc = ctx.enter_context(tile.TileContext(nc))
        const = ctx.enter_context(tc.tile_pool(name="const", bufs=1))
        big = ctx.enter_context(tc.tile_pool(name="big", bufs=1))
        ppool = ctx.enter_context(tc.tile_pool(name="ppool", bufs=4))
        otpool = ctx.enter_context(tc.tile_pool(name="otpool", bufs=2))
        psw = ctx.enter_context(tc.tile_pool(name="psw", bufs=2, space="PSUM"))
        pss2 = ctx.enter_context(
            tc.tile_pool(name="pss2", bufs=2, space="PSUM"))
        out_ps = ctx.enter_context(
            tc.tile_pool(name="out_ps", bufs=2, space="PSUM"))

        # ---- SBUF tiles ----
        W = const.tile([128, OFF_VT], BF16)        # wqk | wv | ident
        bias = const.tile([128, 2], F32)
        xt_sb = big.tile([128, NQ, DC, QW], BF16)
        # B = [q; k] natural PSUM layout; A = [k; q] swapped copy.
        # S tile T0 uses lhsT=A[0:64] (k), rhs=B[0:64] (q);
        # S tile T8 uses lhsT=B[64:128] (k), rhs=A[64:128] (q).
        qkB = big.tile([128, T], BF16)
        qkA = big.tile([128, T], BF16)
        vT2 = big.tile([128, T], BF16)
        v_sb = big.tile([128, NJT, H + 2], BF16)
        out_sb = big.tile([128, NJT, H], F32)
        recip = big.tile([128, NJT], F32)

        xsrc = wx[:, OFF_X:].rearrange(
            "p (q c t) -> p q c t", q=NQ, c=DC)

        # ---- input DMAs: two HWDGE rings balanced so that x quarter 1
        # completes as early as possible (~17.5us) while x quarter 0 and
        # all constants land by ~13us.
        nc.sync.dma_start(out=W[:, 0:OFF_WV], in_=wx[:, 0:OFF_WV])
        nc.scalar.dma_start(out=xt_sb[:, 0, 0:2, :], in_=xsrc[:, 0, 0:2, :])
        nc.sync.dma_start(out=xt_sb[:, 0, 4:6, :], in_=xsrc[:, 0, 4:6, :])
        nc.scalar.dma_start(out=xt_sb[:, 0, 2:4, :], in_=xsrc[:, 0, 2:4, :])
        nc.sync.dma_start(out=xt_sb[:, 0, 6:8, :], in_=xsrc[:, 0, 6:8, :])
        nc.scalar.dma_start(
            out=W[:, OFF_WV:OFF_VT], in_=wx[:, OFF_WV:OFF_VT])
        nc.sync.dma_start(out=bias[:], in_=bias2[:])
        nc.scalar.dma_start(
            out=v_sb[:, :, H:H + 2],
            in_=wx[:, OFF_VT:OFF_X].rearrange("p (t two) -> p t two", two=2))
        # quarter 1 in four pieces so any scheduler-hoisted prep
        # matmul stalls on a quarter-piece, not the whole quarter
        nc.scalar.dma_start(out=xt_sb[:, 1, 0:2, :], in_=xsrc[:, 1, 0:2, :])
        nc.sync.dma_start(out=xt_sb[:, 1, 4:6, :], in_=xsrc[:, 1, 4:6, :])
        nc.scalar.dma_start(out=xt_sb[:, 1, 2:4, :], in_=xsrc[:, 1, 2:4, :])
        nc.sync.dma_start(out=xt_sb[:, 1, 6:8, :], in_=xsrc[:, 1, 6:8, :])
        for q in range(2, NQ):
            nc.scalar.dma_start(
                out=xt_sb[:, q, 0:4, :], in_=xsrc[:, q, 0:4, :])
            nc.sync.dma_start(
                out=xt_sb[:, q, 4:8, :], in_=xsrc[:, q, 4:8, :])

        ident = W[:, OFF_ID:OFF_ID + 128]

        # ---- PE warmup + ACT table preload during the input-DMA window
        warm = const.tile([128, 512], BF16)
        nc.vector.memset(warm[:], 0.0)
        escr = const.tile([128, 2], F32)
        nc.vector.memset(escr[:], 0.0)
        nc.scalar.activation(out=escr[:], in_=escr[:], func=AF.Exp, scale=1.0)
        ps_w = pss2.tile([128, 2 * QW], F32, tag="s")
        for _ in range(8):
            nc.tensor.matmul(ps_w[:, 0:QW], warm[:, 0:128], warm[:],
                             start=True, stop=True)

        # ---- QKV emitters ----
        def emit_qk_group(g, c_lo, c_hi):
            """q/k for t in [512g, 512(g+1)), chunks [c_lo, c_hi)."""
            sl = slice(g * QW, (g + 1) * QW)
            ps = emit_qk_group.ps
            if c_lo == 0:
                ps = emit_qk_group.ps = psw.tile(
                    [128, QW], F32, tag="w", name=f"psqk{g}")
            for c in range(c_lo, c_hi):
                nc.tensor.matmul(
                    ps[:, 0:QW], W[:, c * 128:(c + 1) * 128],
                    xt_sb[:, g, c, :],
                    start=(c == 0), stop=(c == DC - 1),
                )
            if c_hi == DC:
                nc.scalar.add(qkB[:, sl], ps[:, 0:QW], bias[:, 0:1])
                if g == 0:
                    # critical path: build qkA straight from PSUM so it
                    # does not serialize behind the scalar-engine add
                    nc.vector.tensor_scalar(
                        out=qkA[64:128, sl], in0=ps[0:64, 0:QW],
                        scalar1=bias[0:64, 0:1], scalar2=None, op0=ALU.add)
                    nc.vector.tensor_scalar(
                        out=qkA[0:64, sl], in0=ps[64:128, 0:QW],
                        scalar1=bias[64:128, 0:1], scalar2=None, op0=ALU.add)
                else:
                    # swapped copies from SBUF (bf16, fast DVE mode):
                    # q_hi first (needed by S tile T8 rhs at quarter start)
                    nc.vector.tensor_copy(qkA[64:128, sl], qkB[0:64, sl])
                    nc.vector.tensor_copy(qkA[0:64, sl], qkB[64:128, sl])
        emit_qk_group.ps = None

        def emit_v_quarter(g):
            """v^T for t in [512g, 512(g+1)); col-tiled 2x256."""
            lo = g * QW
            ps = psw.tile([128, QW], F32, tag="w", name=f"psv{g}")
            for c in range(DC):
                wv_c = W[:, OFF_WV + c * H:OFF_WV + (c + 1) * H]
                nc.tensor.matmul(
                    ps[0:64, 0:256], wv_c, xt_sb[:, g, c, 0:256],
                    start=(c == 0), stop=(c == DC - 1))
                nc.tensor.matmul(
                    ps[64:128, 0:256], wv_c, xt_sb[:, g, c, 256:512],
                    start=(c == 0), stop=(c == DC - 1))
            nc.vector.tensor_scalar(
                out=vT2[0:64, lo:lo + 256], in0=ps[0:64, 0:256],
                scalar1=bias[0:64, 1:2], scalar2=None, op0=ALU.add)
            nc.vector.tensor_scalar(
                out=vT2[64:128, lo + 256:lo + 512], in0=ps[64:128, 0:256],
                scalar1=bias[64:128, 1:2], scalar2=None, op0=ALU.add)

        def emit_vtrans(g):
            """transpose v^T [64,128] tiles -> natural v tiles for quarter g.

            vT2 rows 0:64 hold t%512 in [0,256) (blocks j2=0,1), rows
            64:128 hold [256,512) (blocks j2=2,3): transpose pairs
            (j2, j2+2) concurrently on PE row tiles T0/T8."""
            lo = g * QW
            ps_a = psw.tile([128, 2, H], BF16, tag="w", name=f"psvta{g}")
            ps_b = psw.tile([128, 2, H], BF16, tag="w", name=f"psvtb{g}")
            for j2 in range(2):
                nc.tensor.transpose(
                    ps_a[:, j2, :],
                    vT2[0:64, lo + j2 * 128:lo + (j2 + 1) * 128],
                    ident[0:64, 0:64])
                nc.tensor.transpose(
                    ps_b[:, j2, :],
                    vT2[64:128, lo + 256 + j2 * 128:lo + 256 + (j2 + 1) * 128],
                    ident[64:128, 64:128])
            nc.vector.tensor_copy(v_sb[:, 4 * g:4 * g + 2, 0:H], ps_a[:, :, :])
            nc.vector.tensor_copy(
                v_sb[:, 4 * g + 2:4 * g + 4, 0:H], ps_b[:, :, :])

        # ---- attention round ----
        def emit_round(q, p, ps_o, n_pairs, drain_fl, prep_fl):
            """pops ~one round of filler work between the S matmuls and
            PV; drains are always data-ready, preps only near quarter
            end (when the next x quarter has landed)."""
            jt0, jt1 = 2 * p, 2 * p + 1
            off0 = max(0, 128 * jt0 - QW * q)
            off1 = max(0, 128 * jt1 - QW * q)
            ps_s = pss2.tile([128, 2 * QW], F32, tag="s")
            hp = tc.high_priority(offset=600)
            hp.__enter__()
            nc.tensor.matmul(
                ps_s[:, off0:QW],
                qkA[0:64, jt0 * 128:(jt0 + 1) * 128],
                qkB[0:64, QW * q + off0:QW * (q + 1)],
                start=True, stop=True)
            nc.tensor.matmul(
                ps_s[:, QW + off1:2 * QW],
                qkB[64:128, jt1 * 128:(jt1 + 1) * 128],
                qkA[64:128, QW * q + off1:QW * (q + 1)],
                start=True, stop=True)
            P = ppool.tile([128, 2 * QW], BF16, tag="P")
            nc.scalar.activation(
                out=P[:, off0:], in_=ps_s[:, off0:], func=AF.Exp, scale=SCALE)
            if jt0 >= 4 * q:
                nc.gpsimd.affine_select(
                    out=P[:, off0:off0 + 128], in_=P[:, off0:off0 + 128],
                    compare_op=ALU.is_ge, fill=0.0,
                    base=0, pattern=[[1, 128]], channel_multiplier=-1)
            if jt1 >= 4 * q:
                nc.gpsimd.affine_select(
                    out=P[:, QW + off1:QW + off1 + 128],
                    in_=P[:, QW + off1:QW + off1 + 128],
                    compare_op=ALU.is_ge, fill=0.0,
                    base=0, pattern=[[1, 128]], channel_multiplier=-1)
            nc.tensor.matmul(
                ps_o[:, off0:QW], v_sb[:, jt0, :], P[:, off0:QW],
                start=(p == 0), stop=False)
            nc.tensor.matmul(
                ps_o[:, off1:QW], v_sb[:, jt1, :], P[:, QW + off1:2 * QW],
                start=False, stop=(p == n_pairs - 1))
            hp.__exit__(None, None, None)
            budget = 0.8
            while drain_fl and budget > 0:
                w, f = drain_fl.pop(0)
                f()
                budget -= w
            if prep_fl is not None:
                while prep_fl and budget > 0:
                    w, f = prep_fl.pop(0)
                    f()
                    budget -= w

        # ---- drain ----
        def drain_closures(q, ps_o):
            state = {}

            def _copy(h):
                def go():
                    if "oT" not in state:
                        state["oT"] = otpool.tile(
                            [66, QW], BF16, tag="oT", name=f"oT{q}")
                    dst = state["oT"][:, h * 256:(h + 1) * 256]
                    srcp = ps_o[:, h * 256:(h + 1) * 256]
                    if q == NQ - 1:
                        nc.scalar.copy(dst, srcp)
                    else:
                        nc.vector.tensor_copy(dst, srcp)
                return go

            def _tr(t2):
                def go():
                    if "psn" not in state:
                        state["psn"] = psw.tile(
                            [128, 4, H + 2], BF16, tag="w", name=f"psn{q}")
                    nc.tensor.transpose(
                        state["psn"][:, t2, 0:66],
                        state["oT"][:, t2 * 128:(t2 + 1) * 128],
                        ident[0:66, 0:66])
                return go

            def _fin(h):
                def go():
                    psn = state["psn"]
                    last = q == NQ - 1
                    sl = slice(q * 4 + 2 * h, q * 4 + 2 * h + 2)
                    nc.vector.reciprocal(
                        recip[:, sl], psn[:, 2 * h:2 * h + 2, H])
                    for t2 in (2 * h, 2 * h + 1):
                        if last and t2 % 2 == 0:
                            nc.scalar.mul(
                                out_sb[:, 4 * q + t2, :], psn[:, t2, 0:H],
                                recip[:, 4 * q + t2:4 * q + t2 + 1])
                        else:
                            nc.vector.tensor_scalar_mul(
                                out_sb[:, 4 * q + t2, :], psn[:, t2, 0:H],
                                recip[:, 4 * q + t2:4 * q + t2 + 1])
                    if last and h == 0:
                        pass  # single combined DMA issued by fin(1)
                    elif last:
                        nc.sync.dma_start(
                            out=out.rearrange(
                                "(qq tt p) h -> qq p tt h", qq=NQ, p=128)[q],
                            in_=out_sb[:, 4 * q:4 * q + 4, :])
                    else:
                        nc.sync.dma_start(
                            out=out.rearrange(
                                "(qq hh p) h -> qq p hh h", qq=2 * NQ, p=128
                            )[2 * q + h],
                            in_=out_sb[:, sl, :])
                return go

            return [(0.25, _copy(0)), (0.2, _tr(0)), (0.2, _tr(1)),
                    (0.25, _copy(1)), (0.2, _tr(2)), (0.2, _tr(3)),
                    (0.3, _fin(0)), (0.3, _fin(1))]

        # ---- emission schedule ----
        # critical path for quarter 0
        emit_qk_group(0, 0, 4)
        emit_qk_group(0, 4, 8)
        # warm-keeping fillers anchored to x0's last chunk: they occupy
        # the PE during the x-quarter-1 wait so the HAM clock gate never
        # re-throttles mid lead-in (they cannot delay the exp chain,
        # which depends on qkA/qkB, not on these)
        for _ in range(4):
            nc.tensor.matmul(ps_w[:, 0:QW], warm[:, 0:128],
                             xt_sb[:, 0, 7, :], start=True, stop=True)
        emit_v_quarter(0)
        emit_vtrans(0)

        def prep_closures(g):
            return [(0.95, lambda: emit_qk_group(g, 0, 4)),
                    (0.95, lambda: emit_qk_group(g, 4, 8)),
                    (1.1, lambda: emit_v_quarter(g)),
                    (0.9, lambda: emit_vtrans(g))]

        drain_fl = []
        for q in range(NQ):
            n_pairs = 2 * (q + 1)
            ps_o = out_ps.tile([H + 2, QW], F32, tag="out", name=f"pso{q}")
            prep_fl = prep_closures(q + 1) if q + 1 < NQ else []
            dq3 = None
            for p in range(n_pairs):
                allow_prep = q >= 1 and p >= n_pairs - 3
                emit_round(q, p, ps_o, n_pairs, drain_fl,
                           prep_fl if allow_prep else None)
                if q == NQ - 1 and p == n_pairs - 2:
                    # out columns [0:256] are final after this round
                    # (jts 14/15 write [256:512] only): overlap the
                    # first oT copy and its two transposes with the
                    # last exp
                    dq3 = drain_closures(q, ps_o)
                    for _ in range(3):
                        w, f = dq3.pop(0)
                        f()
            # leftovers must complete before quarter q+1's rounds
            for w, f in drain_fl:
                f()
            for w, f in prep_fl:
                f()
            drain_fl = dq3 if dq3 is not None else drain_closures(q, ps_o)
        for w, f in drain_fl:
            f()

    nc.compile()
    return nc


def _get_nc():
    if "nc" not in _CACHE:
        _CACHE["nc"] = _build()
    return _CACHE["nc"]


def kernel(x, Wq, bq, Wk, bk, Wv, bv):
    x = np.ascontiguousarray(np.asarray(x, dtype=np.float32))
    Wq = np.asarray(Wq, dtype=np.float32)
    Wk = np.asarray(Wk, dtype=np.float32)
    Wv = np.ascontiguousarray(np.asarray(Wv, dtype=np.float32))
    bq = np.asarray(bq, dtype=np.float32)
    bk = np.asarray(bk, dtype=np.float32)
    bv = np.asarray(bv, dtype=np.float32)

    bf = ml_dtypes.bfloat16
    # wqk: [1024, 128] -> [128p, 8c, 128m]
    wqk = np.concatenate([Wq, Wk], axis=1).reshape(DC, 128, 128)
    wqk = np.transpose(wqk, (1, 0, 2)).reshape(128, DC * 128)
    # wv: [1024, 64] -> [128p, 8c, 64m]
    wv = Wv.reshape(DC, 128, H)
    wv = np.transpose(wv, (1, 0, 2)).reshape(128, DC * H)
    ident = np.eye(128, dtype=np.float32)
    vtail = np.zeros((128, NJT, 2), dtype=np.float32)
    vtail[:, :, 0] = 1.0
    head = np.concatenate(
        [wqk, wv, ident, vtail.reshape(128, 2 * NJT)], axis=1).astype(bf)

    bias2 = np.zeros((128, 2), dtype=np.float32)
    bias2[:, 0] = np.concatenate([bq, bk])
    bias2[:, 1] = np.concatenate([bv, bv])

    in_maps = []
    for b in range(NB):
        # x[b].T: [1024, 2048] -> [128p, 4q, 8c, 512t]
        xt = np.ascontiguousarray(x[b].T).reshape(DC, 128, NQ, QW)
        xt = np.transpose(xt, (1, 2, 0, 3)).reshape(128, DC * T).astype(bf)
        blob = np.concatenate([head, xt], axis=1)
        in_maps.append({
            "wx": np.ascontiguousarray(blob),
            "bias2": bias2,
        })

    nc = _get_nc()
    trace = bool(int(os.environ.get("KTRACE", "0")))
    res = run_bass_kernel_spmd(
        nc, in_maps, core_ids=list(range(NB)), trace=trace,
    )
    if trace:
        _CACHE["exec_time_ns"] = res.exec_time_ns
        _CACHE["results"] = res
    return np.stack([r["out"] for r in res.results])


# revision 41
# speedup vs baseline: 1.0037x; 1.0037x over previous
"""Causal single-head attention on 8 Trainium2 NeuronCores.

Problem: x[8, 2048, 1024] -> out[8, 2048, 64]
  q/k/v = x @ W{q,k,v} + b{q,k,v};  out = softmax(causal(q k^T / 8)) v

Sharding: data-parallel over batch; core b computes batch element b.

Per-core design (T=2048, D=1024, H=64), all matmul operands bf16 with
fp32 PSUM accumulation:
  - host packs ONE bf16 blob [128, 18080] = wqk | wv | ident | vtail | x^T
    plus a [128, 2] f32 bias blob; 8 merged DMA issues (bias, wqk, x
    quarter 0 in two halves, wv+ident+vtail, x quarters 1-3) so the
    first QKV matmul starts as early as possible.
  - QKV per 512-col t-group: lhsT = wqk chunk [128d, 128] -> qT/kT
    [64, T]; V is COL-TILED: two M=64 matmuls at tile_position (0,0)
    and (0,64) compute v^T for the two 256-col halves of a quarter
    concurrently on disjoint PE column groups.
  - v^T tiles PE-transposed in bf16 (cheap) to natural v [128t, 64h]
    + ones/zeros columns -> v_sb [128, 16, 66].
  - attention in jt-PAIR rounds per i-quarter (512 wide): S^T for two
    j-chunks -> one 2-bank PSUM tile [128, 1024]; ONE exp ACTIVATE per
    round (scale=1/8 fused) -> P bf16; causal diagonal blocks masked by
    affine_select; PV accumulates out^T[66, 512] per quarter, whose
    row 64 is the softmax denominator (ones column of v_sb).
  - drain per quarter: out^T cast to bf16, PE-transposed (bf16) to
    natural [128, 4, 66] PSUM, reciprocal of row 64, per-t-tile scale,
    DMA out. Drains and later-quarter QKV work run as fillers inside
    earlier rounds to keep the PE busy during exp waits.
"""

import os
from contextlib import ExitStack

import ml_dtypes
import numpy as np

import concourse.bacc as bacc
import concourse.mybir as mybir
import concourse.tile as tile
from concourse.bass_utils import run_bass_kernel_spmd

F32 = mybir.dt.float32
BF16 = mybir.dt.bfloat16
AF = mybir.ActivationFunctionType
ALU = mybir.AluOpType

T = 2048
D = 1024
H = 64
NB = 8
DC = D // 128       # 8 contraction chunks
NJT = T // 128      # 16 j-chunks (also 16 t-tiles)
QW = 512            # i-quarter width
NQ = T // QW        # 4 quarters
SCALE = 1.0 / 8.0   # 1/sqrt(H)

# blob column offsets (bf16 elements per partition)
OFF_WQK = 0            # [128, 8, 128] -> 1024
OFF_WV = 1024          # [128, 8, 64]  -> 512
OFF_ID = 1536          # [128, 128]    -> 128
OFF_VT = 1664          # [128, 16, 2]  -> 32
OFF_X = 1696           # [128, 8, 2048] -> 16384
BLOB_W = OFF_X + DC * T

_CACHE: dict = {}


def _build():
    nc = bacc.Bacc("TRN2", target_bir_lowering=False, debug=False,
                   num_devices=NB)
    wx = nc.dram_tensor("wx", [128, BLOB_W], BF16, kind="ExternalInput")
    bias2 = nc.dram_tensor("bias2", [128, 2], F32, kind="ExternalInput")
    out = nc.dram_tensor("out", [T, H], F32, kind="ExternalOutput")

    with ExitStack() as ctx:
        tc = ctx.enter_context(tile.TileContext(nc))
        const = ctx.enter_context(tc.tile_pool(name="const", bufs=1))
        big = ctx.enter_context(tc.tile_pool(name="big", bufs=1))
        ppool = ctx.enter_context(tc.tile_pool(name="ppool", bufs=4))
        otpool = ctx.enter_context(tc.tile_pool(name="otpool", bufs=2))
        psw = ctx.enter_context(tc.tile_pool(name="psw", bufs=2, space="PSUM"))
        pss2 = ctx.enter_context(
            tc.tile_pool(name="pss2", bufs=2, space="PSUM"))
        out_ps = ctx.enter_context(
            tc.tile_pool(name="out_ps", bufs=2, space="PSUM"))

        # ---- SBUF tiles ----
        W = const.tile([128, OFF_VT], BF16)        # wqk | wv | ident
        bias = const.tile([128, 2], F32)
        xt_sb = big.tile([128, NQ, DC, QW], BF16)
        # B = [q; k] natural PSUM layout; A = [k; q] swapped copy.
        # S tile T0 uses lhsT=A[0:64] (k), rhs=B[0:64] (q);
        # S tile T8 uses lhsT=B[64:128] (k), rhs=A[64:128] (q).
        qkB = big.tile([128, T], BF16)
        qkA = big.tile([128, T], BF16)
        vT2 = big.tile([128, T], BF16)
        v_sb = big.tile([128, NJT, H + 2], BF16)
        out_sb = big.tile([128, NJT, H], F32)
        recip = big.tile([128, NJT], F32)

        xsrc = wx[:, OFF_X:].rearrange(
            "p (q c t) -> p q c t", q=NQ, c=DC)

        # ---- input DMAs: two HWDGE rings balanced so that x quarter 1
        # completes as early as possible (~17.5us) while x quarter 0 and
        # all constants land by ~13us.
        nc.sync.dma_start(out=W[:, 0:OFF_WV], in_=wx[:, 0:OFF_WV])
        nc.scalar.dma_start(out=xt_sb[:, 0, 0:2, :], in_=xsrc[:, 0, 0:2, :])
        nc.sync.dma_start(out=xt_sb[:, 0, 4:6, :], in_=xsrc[:, 0, 4:6, :])
        nc.scalar.dma_start(out=xt_sb[:, 0, 2:4, :], in_=xsrc[:, 0, 2:4, :])
        nc.sync.dma_start(out=xt_sb[:, 0, 6:8, :], in_=xsrc[:, 0, 6:8, :])
        nc.scalar.dma_start(
            out=W[:, OFF_WV:OFF_VT], in_=wx[:, OFF_WV:OFF_VT])
        nc.sync.dma_start(out=bias[:], in_=bias2[:])
        nc.scalar.dma_start(
            out=v_sb[:, :, H:H + 2],
            in_=wx[:, OFF_VT:OFF_X].rearrange("p (t two) -> p t two", two=2))
        # quarter 1 in four pieces so any scheduler-hoisted prep
        # matmul stalls on a quarter-piece, not the whole quarter
        nc.scalar.dma_start(out=xt_sb[:, 1, 0:2, :], in_=xsrc[:, 1, 0:2, :])
        nc.sync.dma_start(out=xt_sb[:, 1, 4:6, :], in_=xsrc[:, 1, 4:6, :])
        nc.scalar.dma_start(out=xt_sb[:, 1, 2:4, :], in_=xsrc[:, 1, 2:4, :])
        nc.sync.dma_start(out=xt_sb[:, 1, 6:8, :], in_=xsrc[:, 1, 6:8, :])
        for q in range(2, NQ):
            nc.scalar.dma_start(
                out=xt_sb[:, q, 0:4, :], in_=xsrc[:, q, 0:4, :])
            nc.sync.dma_start(
                out=xt_sb[:, q, 4:8, :], in_=xsrc[:, q, 4:8, :])

        ident = W[:, OFF_ID:OFF_ID + 128]

        # ---- PE warmup + ACT table preload during the input-DMA window
        warm = const.tile([128, 512], BF16)
        nc.vector.memset(warm[:], 0.0)
        escr = const.tile([128, 2], F32)
        nc.vector.memset(escr[:], 0.0)
        nc.scalar.activation(out=escr[:], in_=escr[:], func=AF.Exp, scale=1.0)
        ps_w = pss2.tile([128, 2 * QW], F32, tag="s")
        for _ in range(8):
            nc.tensor.matmul(ps_w[:, 0:QW], warm[:, 0:128], warm[:],
                             start=True, stop=True)

        # ---- QKV emitters ----
        def emit_qk_group(g, c_lo, c_hi):
            """q/k for t in [512g, 512(g+1)), chunks [c_lo, c_hi)."""
            sl = slice(g * QW, (g + 1) * QW)
            ps = emit_qk_group.ps
            if c_lo == 0:
                ps = emit_qk_group.ps = psw.tile(
                    [128, QW], F32, tag="w", name=f"psqk{g}")
            for c in range(c_lo, c_hi):
                nc.tensor.matmul(
                    ps[:, 0:QW], W[:, c * 128:(c + 1) * 128],
                    xt_sb[:, g, c, :],
                    start=(c == 0), stop=(c == DC - 1),
                )
            if c_hi == DC:
                nc.scalar.add(qkB[:, sl], ps[:, 0:QW], bias[:, 0:1])
                if g == 0:
                    # critical path: build qkA straight from PSUM so it
                    # does not serialize behind the scalar-engine add
                    nc.vector.tensor_scalar(
                        out=qkA[64:128, sl], in0=ps[0:64, 0:QW],
                        scalar1=bias[0:64, 0:1], scalar2=None, op0=ALU.add)
                    nc.vector.tensor_scalar(
                        out=qkA[0:64, sl], in0=ps[64:128, 0:QW],
                        scalar1=bias[64:128, 0:1], scalar2=None, op0=ALU.add)
                else:
                    # swapped copies from SBUF (bf16, fast DVE mode):
                    # q_hi first (needed by S tile T8 rhs at quarter start)
                    nc.vector.tensor_copy(qkA[64:128, sl], qkB[0:64, sl])
                    nc.vector.tensor_copy(qkA[0:64, sl], qkB[64:128, sl])
        emit_qk_group.ps = None

        def emit_v_quarter(g):
            """v^T for t in [512g, 512(g+1)); col-tiled 2x256."""
            lo = g * QW
            ps = psw.tile([128, QW], F32, tag="w", name=f"psv{g}")
            for c in range(DC):
                wv_c = W[:, OFF_WV + c * H:OFF_WV + (c + 1) * H]
                nc.tensor.matmul(
                    ps[0:64, 0:256], wv_c, xt_sb[:, g, c, 0:256],
                    start=(c == 0), stop=(c == DC - 1))
                nc.tensor.matmul(
                    ps[64:128, 0:256], wv_c, xt_sb[:, g, c, 256:512],
                    start=(c == 0), stop=(c == DC - 1))
            nc.vector.tensor_scalar(
                out=vT2[0:64, lo:lo + 256], in0=ps[0:64, 0:256],
                scalar1=bias[0:64, 1:2], scalar2=None, op0=ALU.add)
            nc.vector.tensor_scalar(
                out=vT2[64:128, lo + 256:lo + 512], in0=ps[64:128, 0:256],
                scalar1=bias[64:128, 1:2], scalar2=None, op0=ALU.add)

        def emit_vtrans(g):
            """transpose v^T [64,128] tiles -> natural v tiles for quarter g.

            vT2 rows 0:64 hold t%512 in [0,256) (blocks j2=0,1), rows
            64:128 hold [256,512) (blocks j2=2,3): transpose pairs
            (j2, j2+2) concurrently on PE row tiles T0/T8."""
            lo = g * QW
            ps_a = psw.tile([128, 2, H], BF16, tag="w", name=f"psvta{g}")
            ps_b = psw.tile([128, 2, H], BF16, tag="w", name=f"psvtb{g}")
            for j2 in range(2):
                nc.tensor.transpose(
                    ps_a[:, j2, :],
                    vT2[0:64, lo + j2 * 128:lo + (j2 + 1) * 128],
                    ident[0:64, 0:64])
                nc.tensor.transpose(
                    ps_b[:, j2, :],
                    vT2[64:128, lo + 256 + j2 * 128:lo + 256 + (j2 + 1) * 128],
                    ident[64:128, 64:128])
            nc.vector.tensor_copy(v_sb[:, 4 * g:4 * g + 2, 0:H], ps_a[:, :, :])
            nc.vector.tensor_copy(
                v_sb[:, 4 * g + 2:4 * g + 4, 0:H], ps_b[:, :, :])

        # ---- attention round ----
        def emit_round(q, p, ps_o, n_pairs, drain_fl, prep_fl):
            """pops ~one round of filler work between the S matmuls and
            PV; drains are always data-ready, preps only near quarter
            end (when the next x quarter has landed)."""
            jt0, jt1 = 2 * p, 2 * p + 1
            off0 = max(0, 128 * jt0 - QW * q)
            off1 = max(0, 128 * jt1 - QW * q)
            ps_s = pss2.tile([128, 2 * QW], F32, tag="s")
            hp = tc.high_priority(offset=600)
            hp.__enter__()
            nc.tensor.matmul(
                ps_s[:, off0:QW],
                qkA[0:64, jt0 * 128:(jt0 + 1) * 128],
                qkB[0:64, QW * q + off0:QW * (q + 1)],
                start=True, stop=True)
            nc.tensor.matmul(
                ps_s[:, QW + off1:2 * QW],
                qkB[64:128, jt1 * 128:(jt1 + 1) * 128],
                qkA[64:128, QW * q + off1:QW * (q + 1)],
                start=True, stop=True)
            P = ppool.tile([128, 2 * QW], BF16, tag="P")
            nc.scalar.activation(
                out=P[:, off0:], in_=ps_s[:, off0:], func=AF.Exp, scale=SCALE)
            if jt0 >= 4 * q:
                nc.gpsimd.affine_select(
                    out=P[:, off0:off0 + 128], in_=P[:, off0:off0 + 128],
                    compare_op=ALU.is_ge, fill=0.0,
                    base=0, pattern=[[1, 128]], channel_multiplier=-1)
            if jt1 >= 4 * q:
                nc.gpsimd.affine_select(
                    out=P[:, QW + off1:QW + off1 + 128],
                    in_=P[:, QW + off1:QW + off1 + 128],
                    compare_op=ALU.is_ge, fill=0.0,
                    base=0, pattern=[[1, 128]], channel_multiplier=-1)
            nc.tensor.matmul(
                ps_o[:, off0:QW], v_sb[:, jt0, :], P[:, off0:QW],
                start=(p == 0), stop=False)
            nc.tensor.matmul(
                ps_o[:, off1:QW], v_sb[:, jt1, :], P[:, QW + off1:2 * QW],
                start=False, stop=(p == n_pairs - 1))
            hp.__exit__(None, None, None)
            budget = 0.8
            while drain_fl and budget > 0:
                w, f = drain_fl.pop(0)
                f()
                budget -= w
            if prep_fl is not None:
                while prep_fl and budget > 0:
                    w, f = prep_fl.pop(0)
                    f()
                    budget -= w

        # ---- drain ----
        def drain_closures(q, ps_o):
            state = {}

            def _copy(h):
                def go():
                    if "oT" not in state:
                        state["oT"] = otpool.tile(
                            [66, QW], BF16, tag="oT", name=f"oT{q}")
                    dst = state["oT"][:, h * 256:(h + 1) * 256]
                    srcp = ps_o[:, h * 256:(h + 1) * 256]
                    if q == NQ - 1:
                        nc.scalar.copy(dst, srcp)
                    else:
                        nc.vector.tensor_copy(dst, srcp)
                return go

            def _tr(t2):
                def go():
                    if "psn" not in state:
                        state["psn"] = psw.tile(
                            [128, 4, H + 2], BF16, tag="w", name=f"psn{q}")
                    nc.tensor.transpose(
                        state["psn"][:, t2, 0:66],
                        state["oT"][:, t2 * 128:(t2 + 1) * 128],
                        ident[0:66, 0:66])
                return go

            def _fin(h):
                def go():
                    psn = state["psn"]
                    last = q == NQ - 1
                    sl = slice(q * 4 + 2 * h, q * 4 + 2 * h + 2)
                    nc.vector.reciprocal(
                        recip[:, sl], psn[:, 2 * h:2 * h + 2, H])
                    for t2 in (2 * h, 2 * h + 1):
                        if last and t2 % 2 == 0:
                            nc.scalar.mul(
                                out_sb[:, 4 * q + t2, :], psn[:, t2, 0:H],
                                recip[:, 4 * q + t2:4 * q + t2 + 1])
                        else:
                            nc.vector.tensor_scalar_mul(
                                out_sb[:, 4 * q + t2, :], psn[:, t2, 0:H],
                                recip[:, 4 * q + t2:4 * q + t2 + 1])
                    if last and h == 0:
                        pass  # single combined DMA issued by fin(1)
                    elif last:
                        nc.sync.dma_start(
                            out=out.rearrange(
                                "(qq tt p) h -> qq p tt h", qq=NQ, p=128)[q],
                            in_=out_sb[:, 4 * q:4 * q + 4, :])
                    else:
                        nc.sync.dma_start(
                            out=out.rearrange(
                                "(qq hh p) h -> qq p hh h", qq=2 * NQ, p=128
                            )[2 * q + h],
                            in_=out_sb[:, sl, :])
                return go

            return [(0.25, _copy(0)), (0.2, _tr(0)), (0.2, _tr(1)),
                    (0.25, _copy(1)), (0.2, _tr(2)), (0.2, _tr(3)),
                    (0.3, _fin(0)), (0.3, _fin(1))]

        # ---- emission schedule ----
        # critical path for quarter 0
        emit_qk_group(0, 0, 4)
        emit_qk_group(0, 4, 8)
        emit_v_quarter(0)
        emit_vtrans(0)

        def prep_closures(g):
            return [(0.95, lambda: emit_qk_group(g, 0, 4)),
                    (0.95, lambda: emit_qk_group(g, 4, 8)),
                    (1.1, lambda: emit_v_quarter(g)),
                    (0.9, lambda: emit_vtrans(g))]

        drain_fl = []
        for q in range(NQ):
            n_pairs = 2 * (q + 1)
            ps_o = out_ps.tile([H + 2, QW], F32, tag="out", name=f"pso{q}")
            prep_fl = prep_closures(q + 1) if q + 1 < NQ else []
            dq3 = None
            for p in range(n_pairs):
                allow_prep = q >= 1 and p >= n_pairs - 3
                emit_round(q, p, ps_o, n_pairs, drain_fl,
                           prep_fl if allow_prep else None)
                if q == NQ - 1 and p == n_pairs - 2:
                    # out columns [0:256] are final after this round
                    # (jts 14/15 write [256:512] only): overlap the
                    # first oT copy and its two transposes with the
                    # last exp
                    dq3 = drain_closures(q, ps_o)
                    for _ in range(3):
                        w, f = dq3.pop(0)
                        f()
            # leftovers must complete before quarter q+1's rounds
            for w, f in drain_fl:
                f()
            for w, f in prep_fl:
                f()
            drain_fl = dq3 if dq3 is not None else drain_closures(q, ps_o)
        for w, f in drain_fl:
            f()

    nc.compile()
    return nc


def _get_nc():
    if "nc" not in _CACHE:
        _CACHE["nc"] = _build()
    return _CACHE["nc"]


def kernel(x, Wq, bq, Wk, bk, Wv, bv):
    x = np.ascontiguousarray(np.asarray(x, dtype=np.float32))
    Wq = np.asarray(Wq, dtype=np.float32)
    Wk = np.asarray(Wk, dtype=np.float32)
    Wv = np.ascontiguousarray(np.asarray(Wv, dtype=np.float32))
    bq = np.asarray(bq, dtype=np.float32)
    bk = np.asarray(bk, dtype=np.float32)
    bv = np.asarray(bv, dtype=np.float32)

    bf = ml_dtypes.bfloat16
    # wqk: [1024, 128] -> [128p, 8c, 128m]
    wqk = np.concatenate([Wq, Wk], axis=1).reshape(DC, 128, 128)
    wqk = np.transpose(wqk, (1, 0, 2)).reshape(128, DC * 128)
    # wv: [1024, 64] -> [128p, 8c, 64m]
    wv = Wv.reshape(DC, 128, H)
    wv = np.transpose(wv, (1, 0, 2)).reshape(128, DC * H)
    ident = np.eye(128, dtype=np.float32)
    vtail = np.zeros((128, NJT, 2), dtype=np.float32)
    vtail[:, :, 0] = 1.0
    head = np.concatenate(
        [wqk, wv, ident, vtail.reshape(128, 2 * NJT)], axis=1).astype(bf)

    bias2 = np.zeros((128, 2), dtype=np.float32)
    bias2[:, 0] = np.concatenate([bq, bk])
    bias2[:, 1] = np.concatenate([bv, bv])

    in_maps = []
    for b in range(NB):
        # x[b].T: [1024, 2048] -> [128p, 4q, 8c, 512t]
        xt = np.ascontiguousarray(x[b].T).reshape(DC, 128, NQ, QW)
        xt = np.transpose(xt, (1, 2, 0, 3)).reshape(128, DC * T).astype(bf)
        blob = np.concatenate([head, xt], axis=1)
        in_maps.append({
            "wx": np.ascontiguousarray(blob),
            "bias2": bias2,
        })

    nc = _get_nc()
    trace = bool(int(os.environ.get("KTRACE", "0")))
    res = run_bass_kernel_spmd(
        nc, in_maps, core_ids=list(range(NB)), trace=trace,
    )
    if trace:
        _CACHE["exec_time_ns"] = res.exec_time_ns
        _CACHE["results"] = res
    return np.stack([r["out"] for r in res.results])
